# revision 1
# baseline (speedup 1.0000x reference)
"""Trainium2 Bass kernel for a 6-layer transformer decoder (B=8, S=512, D=512,
H=8, DK=DV=64, DFF=2048, vocab 32000).

Strategy: data-parallel over the batch — each of the 8 NeuronCores runs the
full decoder stack for one batch element. No collectives needed.

On-device layout: activations are kept transposed, xT[d, s], stored as SBUF
tiles [128, 4, 512] (partition = d % 128, then d-subtile, then s). Matmuls run
on the PE in MM_DT (float32r or bfloat16, fp32 PSUM accumulation). LayerNorm
statistics are computed with PE column-sum matmuls (contraction over
partitions); [1, S] rows are broadcast across partitions with K=1 matmuls and
applied with DVE divides (no slow single-lane reciprocals).

Attention per head pair: scores are computed transposed, scT[sk, sq], in four
128-row sk-chunks, interleaved across the pair so the two K=64 matmuls occupy
disjoint PE row-groups and overlap. Softmax denominators come free from an
appended all-ones column in the value projection (W_v augmented on the host
with the bias row and a ones column); a K=2 selector matmul broadcasts the
pair's two denominator rows across the 128 output partitions in one shot.
Causal masking (detected on the host) restricts matmul column ranges plus one
128x128 triangular elementwise mask per chunk; arbitrary masks fall back to
adding the (pre-scaled) mask via an identity-weight matmul into the scores
PSUM accumulation.
"""

import os
import numpy as np

_CONCOURSE_PATHS = ["/opt/trn_rl_repo", "/root/.axon_site/_ro/trn_rl_repo"]


def _ensure_path():
    try:
        import concourse.bass  # noqa: F401
    except Exception:
        import sys

        for p in _CONCOURSE_PATHS:
            if p not in sys.path and os.path.isdir(p):
                sys.path.insert(0, p)


V, D, NL, DK, DVh, H, DFF = 32000, 512, 6, 64, 64, 8, 2048
B, S = 8, 512
EPS = 1e-5
P = 128
NSUB = D // P  # 4 d-subtiles
NCH = S // P  # 4 s-chunks
NF = DFF // P  # 16 dff-chunks
HW_COLS = H * (DVh + 1)  # 520 augmented-v columns

# Debug knobs (test.py may override before calling kernel()).
N_LAYERS = NL
TAPS = ()  # e.g. ("sa0", "x1_0", "ca0", "x2_0", "ff0")
MM_DT = "f32r"  # "f32r" | "bf16"

# Results of the last kernel() call (for test.py).
LAST_RESULT = None

_BUILD_CACHE = {}


def _pe_table():
    pos = np.arange(S)[:, None].astype(np.float32)
    i = np.arange(0, D, 2).astype(np.float32)
    ang = pos / np.power(10000.0, i / D)
    pe = np.zeros((S, D), dtype=np.float32)
    pe[:, 0::2] = np.sin(ang)
    pe[:, 1::2] = np.cos(ang)
    return pe


def _to_T_tiles(mat):
    """[S, D]-like -> [P, NSUB, S] transposed-tile layout (mat.T chunked)."""
    t = np.ascontiguousarray(np.asarray(mat, np.float32)).T  # [D, S]
    return np.ascontiguousarray(t.reshape(t.shape[0] // P, P, -1).transpose(1, 0, 2))


def _col_layout(vec):
    """[D]-like -> [P, D//P] per-partition column layout."""
    v = np.asarray(vec, np.float32).reshape(-1)
    return np.ascontiguousarray(v.reshape(v.shape[0] // P, P).T)


def _build(n_layers, causal_self, self_needs_mask, cross_needs_mask, taps, mm_dt):
    _ensure_path()
    import concourse.mybir as mybir
    from concourse import bacc
    from concourse.tile import TileContext

    dt = mybir.dt
    AF = mybir.ActivationFunctionType
    OP = mybir.AluOpType
    f32 = dt.float32
    fsb = dt.float32r if mm_dt == "f32r" else dt.bfloat16
    # below 256 moving cols fp32r drops to 1/4 rate; bf16 doesn't
    n_floor = 256 if mm_dt == "f32r" else 0

    nc = bacc.Bacc("TRN2", target_bir_lowering=False, debug=False, num_devices=8)

    def din(name, shape, d=None):
        return nc.dram_tensor(name, shape, d or fsb, kind="ExternalInput")

    x0T_d = din("x0T", [P, NSUB, S])
    peT_d = din("peT", [P, NSUB, S])
    encT_d = din("encT", [P, NSUB, S])
    ones_d = din("ones_row", [1, S])
    invD_d = din("invD_col", [P, 1])
    tri_d = din("tri01", [P, P]) if causal_self else None
    ident_d = din("ident", [P, P]) if (self_needs_mask or cross_needs_mask) else None
    smask_d = din("smaskT8", [P, NCH, S]) if self_needs_mask else None
    cmask_d = din("cmaskT8", [P, NCH, S]) if cross_needs_mask else None

    wq_s_d = din("wq_s", [n_layers, P, NSUB, D])
    wk_s_d = din("wk_s", [n_layers, P, NSUB, D])
    wv_s_d = din("wv_s", [n_layers, P, NSUB, HW_COLS])
    bq_s_d = din("bq_s", [n_layers, P, NSUB], f32)
    bk_s_d = din("bk_s", [n_layers, P, NSUB], f32)
    bv_s_d = din("bv_s", [n_layers, 1, HW_COLS])
    wq_c_d = din("wq_c", [n_layers, P, NSUB, D])
    wk_c_d = din("wk_c", [n_layers, P, NSUB, D])
    wv_c_d = din("wv_c", [n_layers, P, NSUB, HW_COLS])
    bq_c_d = din("bq_c", [n_layers, P, NSUB], f32)
    bk_c_d = din("bk_c", [n_layers, P, NSUB], f32)
    bv_c_d = din("bv_c", [n_layers, 1, HW_COLS])
    w1_d = din("w1", [n_layers, P, NSUB, DFF])
    b1_d = din("b1c", [n_layers, P, NF], f32)
    w2_d = din("w2", [n_layers, P, NF, D])
    b2_d = din("b2c", [n_layers, P, NSUB], f32)
    ln1g_d = din("ln1g", [n_layers, P, NSUB], f32)
    ln1b_d = din("ln1b", [n_layers, P, NSUB], f32)
    ln2g_d = din("ln2g", [n_layers, P, NSUB], f32)
    ln2b_d = din("ln2b", [n_layers, P, NSUB], f32)

    out_d = nc.dram_tensor("out_xT", [P, NSUB, S], f32, kind="ExternalOutput")
    tap_d = {
        t: nc.dram_tensor(f"tap_{t}", [P, NSUB, S], fsb, kind="ExternalOutput")
        for t in taps
    }

    def mm(out, lhsT, rhs, start, stop):
        nc.tensor.matmul(
            out, lhsT, rhs, start=start, stop=stop, skip_group_check=True
        )

    with TileContext(nc) as tc:
        with (
            nc.allow_low_precision(reason="reduced-precision matmul pipeline"),
            tc.tile_pool(name="wts", bufs=3 if mm_dt == "f32r" else 6) as wpool,
            tc.tile_pool(name="small", bufs=14) as spool,
            tc.tile_pool(name="brows", bufs=2) as brpool,
            tc.tile_pool(name="qk", bufs=3 if mm_dt == "f32r" else 4) as qkpool,
            tc.tile_pool(name="v", bufs=2 if mm_dt == "f32r" else 3) as vpool,
            tc.tile_pool(name="exp", bufs=2 if mm_dt == "f32r" else 4) as epool,
            tc.tile_pool(name="attn", bufs=2 if mm_dt == "f32r" else 3) as apool,
            tc.tile_pool(name="x", bufs=2 if mm_dt == "f32r" else 3) as xpool,
            tc.tile_pool(name="xout", bufs=1) as xopool,
            tc.tile_pool(name="sq", bufs=1 if mm_dt == "f32r" else 2) as sqpool,
            tc.tile_pool(name="lnt", bufs=2 if mm_dt == "f32r" else 3) as tpool,
            tc.tile_pool(name="ff", bufs=1) as ffpool,
            tc.tile_pool(name="row", bufs=4 if mm_dt == "f32r" else 8) as rpool,
            tc.tile_pool(name="const", bufs=1) as cpool,
            tc.tile_pool(name="ps", bufs=7, space="PSUM") as pspool,
        ):
            # ---- constants & persistent activations ----
            ones_sb = cpool.tile([1, S], fsb, tag="c_ones")
            nc.sync.dma_start(ones_sb[:], ones_d[:])
            invD_sb = cpool.tile([P, 1], fsb, tag="c_invD")
            nc.sync.dma_start(invD_sb[:], invD_d[:])
            if causal_self:
                tri_sb = cpool.tile([P, P], fsb, tag="c_tri")
                nc.sync.dma_start(tri_sb[:], tri_d[:])
            if ident_d is not None:
                id_sb = cpool.tile([P, P], fsb, tag="c_id")
                nc.sync.dma_start(id_sb[:], ident_d[:])
            smask_sb = None
            if self_needs_mask:
                smask_sb = cpool.tile([P, NCH, S], fsb, tag="c_smask")
                nc.sync.dma_start(smask_sb[:], smask_d[:])
            cmask_sb = None
            if cross_needs_mask:
                cmask_sb = cpool.tile([P, NCH, S], fsb, tag="c_cmask")
                nc.sync.dma_start(cmask_sb[:], cmask_d[:])

            encT = cpool.tile([P, NSUB, S], fsb, tag="c_enc")
            nc.sync.dma_start(encT[:], encT_d[:])

            # x0 = emb rows (host-gathered) + positional encoding
            x0r = xpool.tile([P, NSUB, S], fsb, tag="x")
            nc.sync.dma_start(x0r[:], x0T_d[:])
            peT_sb = sqpool.tile([P, NSUB, S], fsb, tag="sq")
            nc.sync.dma_start(peT_sb[:], peT_d[:])
            xT = xpool.tile([P, NSUB, S], fsb, tag="x")
            for i in range(NSUB):
                nc.vector.tensor_tensor(
                    xT[:, i, :], x0r[:, i, :], peT_sb[:, i, :], OP.add
                )

            _psn = [0]

            def ps_tile(n=S, p=P):
                _psn[0] += 1
                return pspool.tile([p, n], f32, tag="ps", name=f"ps{_psn[0]}")

            def proj_T(w_sb, b_sb, srcT, on_act):
                """dk_all x S projection, transposed output [P, NSUB, S]."""
                t = qkpool.tile([P, NSUB, S], fsb, tag="qk")
                for j in range(NSUB):
                    ps = ps_tile()
                    for i in range(NSUB):
                        mm(
                            ps[:],
                            w_sb[:, i, j * P : (j + 1) * P],
                            srcT[:, i, :],
                            start=(i == 0),
                            stop=(i == NSUB - 1),
                        )
                    if on_act:
                        nc.scalar.activation(
                            t[:, j, :], ps[:], AF.Identity, bias=b_sb[:, j : j + 1]
                        )
                    else:
                        nc.vector.tensor_scalar(
                            t[:, j, :], ps[:], b_sb[:, j : j + 1], None, OP.add
                        )
                return t

            def v_aug(w_sb, brow_sb, srcT):
                """augmented v, natural orientation: [P(s), NCH, 520]."""
                vt = vpool.tile([P, NCH, HW_COLS], fsb, tag="v")
                half = HW_COLS // 2  # 260
                for sc in range(NCH):
                    for hh in range(2):
                        cs, ce = hh * half, (hh + 1) * half
                        ps = ps_tile(n=half)
                        for i in range(NSUB):
                            mm(
                                ps[:],
                                srcT[:, i, sc * P : (sc + 1) * P],
                                w_sb[:, i, cs:ce],
                                start=(i == 0),
                                stop=False,
                            )
                        mm(
                            ps[:],
                            ones_sb[0:1, 0:P],
                            brow_sb[0:1, cs:ce],
                            start=False,
                            stop=True,
                        )
                        nc.vector.tensor_copy(vt[:, sc, cs:ce], ps[:])
                return vt

            def attention_core(qT, kT, vt, attnT, causal, mask_sb):
                """Head pairs (2j, 2j+1): interleaved K=64 scores on disjoint
                PE row-groups, AV with fused denominator row, K=2 selector
                broadcast, DVE divide."""
                for j in range(NSUB):
                    exs = [
                        epool.tile([P, NCH, S], fsb, tag="exp", name=f"ex{j}_0"),
                        epool.tile([P, NCH, S], fsb, tag="exp", name=f"ex{j}_1"),
                    ]
                    avs = [ps_tile(), ps_tile()]
                    for c in range(NCH):
                        q0 = c * P if causal else 0
                        qs = min(q0, S - n_floor) if causal else 0
                        scs = [ps_tile(), ps_tile()]
                        for u in range(2):
                            ph = u * 64
                            mm(
                                scs[u][:, qs:S],
                                kT[ph : ph + 64, j, c * P : (c + 1) * P],
                                qT[ph : ph + 64, j, qs:S],
                                start=True,
                                stop=(mask_sb is None),
                            )
                            if mask_sb is not None:
                                mm(
                                    scs[u][:, qs:S],
                                    id_sb[:],
                                    mask_sb[:, c, qs:S],
                                    start=False,
                                    stop=True,
                                )
                        for u in range(2):
                            nc.scalar.activation(
                                exs[u][:, c, q0:S], scs[u][:, q0:S], AF.Exp,
                                scale=0.125,
                            )
                            if causal:
                                nc.vector.tensor_tensor(
                                    exs[u][:, c, c * P : (c + 1) * P],
                                    exs[u][:, c, c * P : (c + 1) * P],
                                    tri_sb[:],
                                    OP.mult,
                                )
                    for c in range(NCH):
                        q0 = c * P if causal else 0
                        for u in range(2):
                            h = 2 * j + u
                            mm(
                                avs[u][0:65, q0:S],
                                vt[:, c, h * 65 : (h + 1) * 65],
                                exs[u][:, c, q0:S],
                                start=(c == 0),
                                stop=(c == NCH - 1),
                            )
                    # per-head: broadcast the denominator row, fast
                    # approximate reciprocal (DVE has no divide), multiply
                    for u in range(2):
                        rsu = rpool.tile([1, S], fsb, tag="row", name=f"rs{j}_{u}")
                        nc.scalar.activation(rsu[:], avs[u][64:65, :], AF.Copy)
                        rb_ps = ps_tile(p=64)
                        mm(rb_ps[:], ones_sb[0:1, 0:64], rsu[0:1, :],
                           start=True, stop=True)
                        rb_sb = rpool.tile([64, S], f32, tag="row",
                                           name=f"rb{j}_{u}")
                        nc.scalar.activation(rb_sb[:], rb_ps[:], AF.Copy)
                        rcp = rpool.tile([64, S], f32, tag="row",
                                         name=f"rc{j}_{u}")
                        nc.vector.reciprocal_approx_fast(rcp[:], rb_sb[:])
                        nc.vector.tensor_tensor(
                            attnT[u * 64 : u * 64 + 64, j, :],
                            avs[u][0:64, :],
                            rcp[:],
                            OP.mult,
                        )

            def layer_norm(x_in, g_sb, b_sb, out_f32=False):
                """LN over partitions (d), per-token stats via PE sums."""
                mean_ps = ps_tile(p=1)
                s2_ps = ps_tile(p=1)
                sq = sqpool.tile([P, NSUB, S], fsb, tag="sq")
                for i in range(NSUB):
                    nc.scalar.activation(sq[:, i, :], x_in[:, i, :], AF.Square)
                for i in range(NSUB):
                    mm(
                        mean_ps[:],
                        invD_sb[:],
                        x_in[:, i, :],
                        start=(i == 0),
                        stop=(i == NSUB - 1),
                    )
                for i in range(NSUB):
                    mm(
                        s2_ps[:],
                        invD_sb[:],
                        sq[:, i, :],
                        start=(i == 0),
                        stop=(i == NSUB - 1),
                    )
                mean_sb = rpool.tile([1, S], fsb, tag="row", name="mean")
                nc.scalar.activation(mean_sb[:], mean_ps[:], AF.Copy)
                msq_sb = rpool.tile([1, S], f32, tag="row", name="msq")
                nc.vector.tensor_tensor(msq_sb[:], mean_ps[:], mean_sb[:], OP.mult)
                var_sb = rpool.tile([1, S], f32, tag="row", name="var")
                nc.vector.tensor_tensor(var_sb[:], s2_ps[:], msq_sb[:], OP.subtract)
                nc.vector.tensor_scalar(var_sb[:], var_sb[:], float(EPS), None, OP.add)
                sd_sb = rpool.tile([1, S], f32, tag="row", name="sd")
                nc.scalar.activation(sd_sb[:], var_sb[:], AF.Sqrt)
                rsd_f = rpool.tile([1, S], f32, tag="row", name="rsdf")
                nc.vector.reciprocal_approx_fast(rsd_f[:], sd_sb[:])
                rsd = rpool.tile([1, S], fsb, tag="row", name="rsd")
                nc.scalar.activation(rsd[:], rsd_f[:], AF.Copy)
                mb_ps = ps_tile()
                mm(mb_ps[:], ones_sb[0:1, 0:P], mean_sb[0:1, :], start=True, stop=True)
                sdb_ps = ps_tile()
                mm(sdb_ps[:], ones_sb[0:1, 0:P], rsd[0:1, :], start=True, stop=True)
                xo = (xopool if out_f32 else xpool).tile(
                    [P, NSUB, S],
                    f32 if out_f32 else fsb,
                    tag="xo" if out_f32 else "x",
                )
                for i in range(NSUB):
                    t1 = tpool.tile([P, S], f32, tag="lnt")
                    nc.vector.tensor_tensor(t1[:], x_in[:, i, :], mb_ps[:], OP.subtract)
                    nc.vector.tensor_tensor(t1[:], t1[:], sdb_ps[:], OP.mult)
                    nc.scalar.activation(
                        xo[:, i, :],
                        t1[:],
                        AF.Identity,
                        bias=b_sb[:, i : i + 1],
                        scale=g_sb[:, i : i + 1],
                    )
                return xo

            def residual(a_T, b_T):
                xo = xpool.tile([P, NSUB, S], fsb, tag="x")
                for i in range(NSUB):
                    nc.vector.tensor_tensor(
                        xo[:, i, :], a_T[:, i, :], b_T[:, i, :], OP.add
                    )
                return xo

            def load_w(src, l, shape):
                t = wpool.tile(shape, fsb, tag="wt")
                nc.sync.dma_start(t[:], src[l])
                return t

            def load_small(src, l, shape, tag):
                if tag == "brow":
                    t = brpool.tile(shape, fsb, tag=tag)
                else:
                    t = spool.tile(shape, f32, tag=tag)
                nc.sync.dma_start(t[:], src[l])
                return t

            def tap(name, tile_):
                if name in tap_d:
                    nc.sync.dma_start(tap_d[name][:], tile_[:])

            for l in range(n_layers):
                # ---- self attention ----
                wq = load_w(wq_s_d, l, [P, NSUB, D])
                wk = load_w(wk_s_d, l, [P, NSUB, D])
                wv = load_w(wv_s_d, l, [P, NSUB, HW_COLS])
                bq = load_small(bq_s_d, l, [P, NSUB], "bcol")
                bk = load_small(bk_s_d, l, [P, NSUB], "bcol")
                bv = load_small(bv_s_d, l, [1, HW_COLS], "brow")
                qT = proj_T(wq, bq, xT, on_act=True)
                kT = proj_T(wk, bk, xT, on_act=False)
                vt = v_aug(wv, bv, xT)
                saT = apool.tile([P, NSUB, S], fsb, tag="attn")
                attention_core(qT, kT, vt, saT, causal_self, smask_sb)
                tap(f"sa{l}", saT)

                # cross K/V from the encoder — independent of LN1, emitted here
                # so the PE has work while LN1's vector chain runs
                wkc = load_w(wk_c_d, l, [P, NSUB, D])
                wvc = load_w(wv_c_d, l, [P, NSUB, HW_COLS])
                bkc = load_small(bk_c_d, l, [P, NSUB], "bcol")
                bvc = load_small(bv_c_d, l, [1, HW_COLS], "brow")
                kcT = proj_T(wkc, bkc, encT, on_act=False)
                vc = v_aug(wvc, bvc, encT)

                g1 = load_small(ln1g_d, l, [P, NSUB], "bcol")
                b1c_ln = load_small(ln1b_d, l, [P, NSUB], "bcol")
                x1 = layer_norm(residual(xT, saT), g1, b1c_ln)
                tap(f"x1_{l}", x1)

                # ---- cross attention ----
                wqc = load_w(wq_c_d, l, [P, NSUB, D])
                bqc = load_small(bq_c_d, l, [P, NSUB], "bcol")
                qcT = proj_T(wqc, bqc, x1, on_act=True)
                caT = apool.tile([P, NSUB, S], fsb, tag="attn")
                attention_core(qcT, kcT, vc, caT, False, cmask_sb)
                tap(f"ca{l}", caT)
                g2 = load_small(ln2g_d, l, [P, NSUB], "bcol")
                b2c_ln = load_small(ln2b_d, l, [P, NSUB], "bcol")
                x2 = layer_norm(residual(x1, caT), g2, b2c_ln)
                tap(f"x2_{l}", x2)

                # ---- FFN ----
                b1col = load_small(b1_d, l, [P, NF], "b1col")
                ff1 = ffpool.tile([P, NF, S], fsb, tag="ff1")
                for g in range(4):  # w1 granules of 512 dff cols
                    w1g = wpool.tile([P, NSUB, 512], fsb, tag="wt")
                    nc.sync.dma_start(
                        w1g[:], w1_d[l, :, :, g * 512 : (g + 1) * 512]
                    )
                    for fl in range(4):
                        F = g * 4 + fl
                        ps = ps_tile()
                        for i in range(NSUB):
                            mm(
                                ps[:],
                                w1g[:, i, fl * P : (fl + 1) * P],
                                x2[:, i, :],
                                start=(i == 0),
                                stop=(i == NSUB - 1),
                            )
                        nc.scalar.activation(
                            ff1[:, F, :], ps[:], AF.Relu, bias=b1col[:, F : F + 1]
                        )
                b2col = load_small(b2_d, l, [P, NSUB], "bcol")
                ffo = apool.tile([P, NSUB, S], fsb, tag="attn")
                for j in range(NSUB):
                    w2g = wpool.tile([P, NF, P], fsb, tag="wt")
                    nc.sync.dma_start(w2g[:], w2_d[l, :, :, j * P : (j + 1) * P])
                    ps = ps_tile()
                    for k in range(NF):
                        mm(
                            ps[:],
                            w2g[:, k, :],
                            ff1[:, k, :],
                            start=(k == 0),
                            stop=(k == NF - 1),
                        )
                    nc.scalar.activation(
                        ffo[:, j, :], ps[:], AF.Identity, bias=b2col[:, j : j + 1]
                    )
                tap(f"ff{l}", ffo)
                xT = layer_norm(
                    residual(x2, ffo), g2, b2c_ln, out_f32=(l == n_layers - 1)
                )

            nc.sync.dma_start(out_d[:], xT[:])

    nc.compile()
    return nc


STAGE = "full"  # debug: "proj" | "attn" | "ln1" | "full"


def _build_fast(n_layers, taps, mm_dt, stage="full"):
    """Fast path. Assumes (host-verified): causal self mask, zero cross mask,
    zero q/k/v/ffn biases, LN gamma == 1, LN beta == 0.

    Single ACT table set (exp/ln/copy/relu/square) — no Sqrt, no table
    thrash. LN rstd = exp(-0.5*ln(var+eps)). Softmax denominators: ones
    column in V (memset once per tile), per-head row reciprocal on DVE,
    K=2 selector matmul broadcast. Multi-bank PSUM tiles with wide
    evacuations, split between ACT (projection copies, relu) and DVE
    (casts, residuals, LN applies) to balance the two engines.
    """
    _ensure_path()
    import concourse.mybir as mybir
    from concourse import bacc
    from concourse.tile import TileContext

    dt = mybir.dt
    AF = mybir.ActivationFunctionType
    OP = mybir.AluOpType
    f32 = dt.float32
    fsb = dt.float32r if mm_dt == "f32r" else dt.bfloat16

    nc = bacc.Bacc("TRN2", target_bir_lowering=False, debug=False, num_devices=8)

    def din(name, shape, d=None):
        return nc.dram_tensor(name, shape, d or fsb, kind="ExternalInput")

    x0T_d = din("x0T", [P, NSUB, S])
    encT_d = din("encT", [P, NSUB, S])
    ones_d = din("ones_row", [1, S])
    invD_d = din("invD_col", [P, 1])
    tri_d = din("tri01", [P, P])
    vones_d = din("vones", [P, NCH, H, 1])

    wq_s_d = din("wq_s", [n_layers, P, NSUB, D])
    wk_s_d = din("wk_s", [n_layers, P, NSUB, D])
    wv_s_d = din("wv_s", [n_layers, P, NSUB, D])
    wq_c_d = din("wq_c", [n_layers, P, NSUB, D])
    wk_c_d = din("wk_c", [n_layers, P, NSUB, D])
    wv_c_d = din("wv_c", [n_layers, P, NSUB, D])
    w1_d = din("w1", [n_layers, P, NSUB, DFF])
    w2_d = din("w2", [n_layers, P, NF, D])

    out_d = nc.dram_tensor("out_xT", [P, NSUB, S], f32, kind="ExternalOutput")
    tap_d = {
        t: nc.dram_tensor(f"tap_{t}", [P, NSUB, S], fsb, kind="ExternalOutput")
        for t in taps
    }

    def mm(out, lhsT, rhs, start, stop):
        nc.tensor.matmul(
            out, lhsT, rhs, start=start, stop=stop, skip_group_check=True
        )

    with TileContext(nc) as tc:
        with (
            nc.allow_low_precision(reason="reduced-precision matmul pipeline"),
            tc.tile_pool(name="wts", bufs=3 if mm_dt == "f32r" else 6) as wpool,
            tc.tile_pool(name="qk", bufs=3 if mm_dt == "f32r" else 4) as qkpool,
            tc.tile_pool(name="v", bufs=2) as vpool,
            tc.tile_pool(name="exp", bufs=2 if mm_dt == "f32r" else 4) as epool,
            tc.tile_pool(name="attn", bufs=2) as apool,
            tc.tile_pool(name="x", bufs=2 if mm_dt == "f32r" else 3) as xpool,
            tc.tile_pool(name="xout", bufs=1) as xopool,
            tc.tile_pool(name="sq", bufs=2) as sqpool,
            tc.tile_pool(name="bc", bufs=2) as bcpool,
            tc.tile_pool(name="rb", bufs=1 if mm_dt == "f32r" else 2) as rbpool,
            tc.tile_pool(name="ff", bufs=1) as ffpool,
            tc.tile_pool(name="row", bufs=6 if mm_dt == "f32r" else 10) as rpool,
            tc.tile_pool(name="const", bufs=1) as cpool,
            tc.tile_pool(name="psA", bufs=3, space="PSUM") as psA,
            tc.tile_pool(name="psB", bufs=2, space="PSUM") as psB,
        ):
            # ---- constants & persistent activations ----
            ones_sb = cpool.tile([1, S], fsb, tag="c_ones")
            nc.sync.dma_start(ones_sb[:], ones_d[:])
            invD_sb = cpool.tile([P, 1], fsb, tag="c_invD")
            nc.sync.dma_start(invD_sb[:], invD_d[:])
            tri_sb = cpool.tile([P, P], fsb, tag="c_tri")
            nc.sync.dma_start(tri_sb[:], tri_d[:])
            encT = cpool.tile([P, NSUB, S], fsb, tag="c_enc")
            nc.sync.dma_start(encT[:], encT_d[:])

            xT = xpool.tile([P, NSUB, S], fsb, tag="x")
            nc.sync.dma_start(xT[:], x0T_d[:])

            def load_w(src, l, shape):
                t = wpool.tile(shape, fsb, tag="wt")
                nc.sync.dma_start(t[:], src[l])
                return t

            def proj_pair(w_sb, srcT):
                """dk_all x S projection -> transposed [P, NSUB, S].
                2-bank PSUM tiles; evacuation via wide ACT copies."""
                t = qkpool.tile([P, NSUB, S], fsb, tag="qk")
                for h in range(2):
                    ps = psA.tile([P, 2, S], f32, tag="ps")
                    for jj in range(2):
                        j = 2 * h + jj
                        for i in range(NSUB):
                            mm(
                                ps[:, jj, :],
                                w_sb[:, i, j * P : (j + 1) * P],
                                srcT[:, i, :],
                                start=(i == 0),
                                stop=(i == NSUB - 1),
                            )
                    nc.scalar.activation(
                        t[:, 2 * h : 2 * h + 2, :], ps[:], AF.Copy
                    )
                return t

            def v_nat(w_sb, srcT):
                """V in natural orientation [P(s), NCH, 8, 65]; v-cols from
                PE, per-head ones column (col 64) memset once."""
                vt = vpool.tile([P, NCH, H, DVh + 1], fsb, tag="v")
                nc.sync.dma_start(vt[:, :, :, DVh : DVh + 1], vones_d[:])
                for h in range(2):
                    ps = psA.tile([P, 2, H, DVh], f32, tag="ps")
                    for cc in range(2):
                        sc = 2 * h + cc
                        for i in range(NSUB):
                            mm(
                                ps[:, cc, :, :],
                                srcT[:, i, sc * P : (sc + 1) * P],
                                w_sb[:, i, :],
                                start=(i == 0),
                                stop=(i == NSUB - 1),
                            )
                    nc.vector.tensor_copy(
                        vt[:, 2 * h : 2 * h + 2, :, 0:DVh], ps[:]
                    )
                return vt

            def attention_core(qT, kT, vt, attnT, causal, fillers):
                """fillers: list of emission thunks interleaved between head
                pairs to give the PE independent work during exp waits."""
                fi = 0
                sub = stage[4:] if stage.startswith("attn") and len(stage) > 4 else ""
                for j in range(NSUB):
                    exs = [
                        epool.tile([P, NCH, S], fsb, tag="exp", name=f"ex{j}_0"),
                        epool.tile([P, NCH, S], fsb, tag="exp", name=f"ex{j}_1"),
                    ]
                    av = [
                        psB.tile([P, S], f32, tag="av", name=f"av{j}_0"),
                        psB.tile([P, S], f32, tag="av", name=f"av{j}_1"),
                    ]
                    for c in range(NCH):
                        q0 = c * P if causal else 0
                        qs = min(q0, S - 256) if causal else 0
                        scp = psA.tile([P, 2, S], f32, tag="ps")
                        for u in range(2):
                            # u = head-pair member; lhsT row groups 0-63 /
                            # 64-127 run concurrently, one bank per head.
                            mm(
                                scp[:, u, qs:S],
                                kT[u * 64 : u * 64 + 64, j, c * P : (c + 1) * P],
                                qT[u * 64 : u * 64 + 64, j, qs:S],
                                start=True,
                                stop=True,
                            )
                        for u in range(2):
                            nc.scalar.activation(
                                exs[u][:, c, q0:S], scp[:, u, q0:S],
                                AF.Exp, scale=0.125,
                            )
                            if causal:
                                nc.vector.tensor_tensor(
                                    exs[u][:, c, c * P : (c + 1) * P],
                                    exs[u][:, c, c * P : (c + 1) * P],
                                    tri_sb[:],
                                    OP.mult,
                                )
                    if sub == "1":  # scores+exp only
                        for u in range(2):
                            nc.vector.tensor_copy(
                                attnT[u * 64 : u * 64 + 64, j, :],
                                exs[u][0:64, 0, :],
                            )
                        continue
                    for c in range(NCH):
                        q0 = c * P if causal else 0
                        for u in range(2):
                            h = 2 * j + u
                            mm(
                                av[u][0 : DVh + 1, q0:S],
                                vt[:, c, h, :],
                                exs[u][:, c, q0:S],
                                start=(c == 0),
                                stop=(c == NCH - 1),
                            )
                    if sub == "2":  # + AV, no denominator
                        for u in range(2):
                            nc.vector.tensor_copy(
                                attnT[u * 64 : u * 64 + 64, j, :],
                                av[u][0:DVh, :],
                            )
                        continue
                    # denominator: per-head row reciprocal -> K=1 broadcast
                    # to 64 partitions (one bank per head, base partition 0)
                    rb_sb = rbpool.tile([64, 2, S], fsb, tag="rb")
                    for u in range(2):
                        d_sb = rpool.tile([1, S], f32, tag="row",
                                          name=f"dn{j}_{u}")
                        nc.scalar.activation(
                            d_sb[:], av[u][DVh : DVh + 1, :], AF.Copy
                        )
                        r_f = rpool.tile([1, S], f32, tag="row",
                                         name=f"rd{j}_{u}")
                        nc.vector.reciprocal_approx_fast(r_f[:], d_sb[:])
                        r_b = rpool.tile([1, S], fsb, tag="row",
                                         name=f"rdb{j}_{u}")
                        nc.vector.tensor_copy(r_b[:], r_f[:])
                        r_rhs = r_b[:]
                        rb_ps = psA.tile([64, S], f32, tag="ps",
                                         name=f"rbp{j}_{u}")
                        mm(
                            rb_ps[:],
                            ones_sb[0:1, 0:64],
                            r_rhs,
                            start=True,
                            stop=True,
                        )
                        nc.vector.tensor_copy(rb_sb[:, u, :], rb_ps[:])
                    for u in range(2):
                        nc.vector.tensor_tensor(
                            attnT[u * 64 : u * 64 + 64, j, :],
                            av[u][0:DVh, :],
                            rb_sb[:, u, :],
                            OP.mult,
                        )
                    if fi < len(fillers):
                        fillers[fi]()
                        fi += 1
                for f in fillers[fi:]:
                    f()

            def layer_norm(y, out_f32=False):
                """LN over partitions (d); gamma==1, beta==0.
                rstd = exp(-0.5 * ln(var + eps)) — same ACT table set."""
                sq = sqpool.tile([P, NSUB, S], fsb, tag="sq")
                nc.vector.tensor_tensor(sq[:], y[:], y[:], OP.mult)
                rows = psA.tile([1, 2, S], f32, tag="ps")
                for i in range(NSUB):
                    mm(rows[0:1, 0, :], invD_sb[:], y[:, i, :],
                       start=(i == 0), stop=(i == NSUB - 1))
                for i in range(NSUB):
                    mm(rows[0:1, 1, :], invD_sb[:], sq[:, i, :],
                       start=(i == 0), stop=(i == NSUB - 1))
                mean_sb = rpool.tile([1, S], fsb, tag="row", name="mean")
                nc.scalar.activation(mean_sb[:], rows[0:1, 0, :], AF.Copy)
                msq_sb = rpool.tile([1, S], f32, tag="row", name="msq")
                nc.scalar.activation(msq_sb[:], rows[0:1, 0, :], AF.Square)
                var_sb = rpool.tile([1, S], f32, tag="row", name="var")
                nc.vector.scalar_tensor_tensor(
                    var_sb[:], rows[0:1, 1, :], float(EPS), msq_sb[:],
                    OP.add, OP.subtract,
                )
                lnv_sb = rpool.tile([1, S], f32, tag="row", name="lnv")
                nc.scalar.activation(lnv_sb[:], var_sb[:], AF.Ln)
                rstd_sb = rpool.tile([1, S], fsb, tag="row", name="rstd")
                nc.scalar.activation(rstd_sb[:], lnv_sb[:], AF.Exp, scale=-0.5)
                bc_ps = psA.tile([P, 2, S], f32, tag="ps")
                mm(bc_ps[:, 0, :], ones_sb[0:1, 0:P], mean_sb[:],
                   start=True, stop=True)
                mm(bc_ps[:, 1, :], ones_sb[0:1, 0:P], rstd_sb[:],
                   start=True, stop=True)
                bc_sb = bcpool.tile([P, 2, S], fsb, tag="bc")
                nc.vector.tensor_copy(bc_sb[:], bc_ps[:])
                t1 = sqpool.tile([P, NSUB, S], fsb, tag="sq")
                xo = (xopool if out_f32 else xpool).tile(
                    [P, NSUB, S], f32 if out_f32 else fsb,
                    tag="xo" if out_f32 else "x",
                )
                for i in range(NSUB):
                    nc.vector.tensor_tensor(
                        t1[:, i, :], y[:, i, :], bc_sb[:, 0, :], OP.subtract
                    )
                    nc.vector.tensor_tensor(
                        xo[:, i, :], t1[:, i, :], bc_sb[:, 1, :], OP.mult
                    )
                return xo

            def residual(a_T, b_T):
                y = xpool.tile([P, NSUB, S], fsb, tag="x")
                nc.vector.tensor_tensor(y[:], a_T[:], b_T[:], OP.add)
                return y

            def tap(name, tile_):
                if name in tap_d:
                    nc.sync.dma_start(tap_d[name][:], tile_[:])

            def ffn_block(l, x2, w1g):
                ff1 = ffpool.tile([P, NF, S], fsb, tag="ff1")
                for g in range(4):
                    for hf in range(2):
                        ps = psA.tile([P, 2, S], f32, tag="ps")
                        for ff in range(2):
                            F = g * 4 + hf * 2 + ff
                            for i in range(NSUB):
                                mm(
                                    ps[:, ff, :],
                                    w1g[g][:, i, (hf * 2 + ff) * P : (hf * 2 + ff + 1) * P],
                                    x2[:, i, :],
                                    start=(i == 0),
                                    stop=(i == NSUB - 1),
                                )
                        F0 = g * 4 + hf * 2
                        nc.scalar.activation(
                            ff1[:, F0 : F0 + 2, :], ps[:], AF.Relu
                        )
                y3 = xpool.tile([P, NSUB, S], fsb, tag="x")
                for jh in range(2):
                    ps = psA.tile([P, 2, S], f32, tag="ps")
                    for jj in range(2):
                        j = 2 * jh + jj
                        w2g = wpool.tile([P, NF, P], fsb, tag="wt")
                        nc.sync.dma_start(
                            w2g[:], w2_d[l, :, :, j * P : (j + 1) * P]
                        )
                        for k in range(NF):
                            mm(
                                ps[:, jj, :],
                                w2g[:, k, :],
                                ff1[:, k, :],
                                start=(k == 0),
                                stop=(k == NF - 1),
                            )
                    nc.vector.tensor_tensor(
                        y3[:, 2 * jh : 2 * jh + 2, :],
                        x2[:, 2 * jh : 2 * jh + 2, :],
                        ps[:],
                        OP.add,
                    )
                return y3

            for l in range(n_layers):
                # ---- self attention ----
                wq = load_w(wq_s_d, l, [P, NSUB, D])
                wk = load_w(wk_s_d, l, [P, NSUB, D])
                wv = load_w(wv_s_d, l, [P, NSUB, D])
                qT = proj_pair(wq, xT)
                kT = proj_pair(wk, xT)
                vt = v_nat(wv, xT)
                if stage == "proj":
                    xT = qT
                    break
                wkc = load_w(wk_c_d, l, [P, NSUB, D])
                wvc = load_w(wv_c_d, l, [P, NSUB, D])
                saT = apool.tile([P, NSUB, S], fsb, tag="attn")
                # cross K/V projections emitted as fillers inside the
                # self-attention head loop — independent PE work that
                # covers the exp/denominator stalls.
                kvc = {}

                def mk_kc():
                    kvc["kcT"] = proj_pair(wkc, encT)

                def mk_vc():
                    kvc["vc"] = v_nat(wvc, encT)

                attention_core(qT, kT, vt, saT, True, [mk_kc, mk_vc])
                tap(f"sa{l}", saT)
                if stage.startswith("attn"):
                    xT = saT
                    break

                x1 = layer_norm(residual(xT, saT))
                tap(f"x1_{l}", x1)
                if stage == "ln1":
                    xT = x1
                    break

                # ---- cross attention ----
                wqc = load_w(wq_c_d, l, [P, NSUB, D])
                qcT = proj_pair(wqc, x1)
                caT = apool.tile([P, NSUB, S], fsb, tag="attn")
                w1g = []

                def mk_w1(g):
                    def f():
                        t = wpool.tile([P, NSUB, 512], fsb, tag="wt")
                        nc.sync.dma_start(
                            t[:], w1_d[l, :, :, g * 512 : (g + 1) * 512]
                        )
                        w1g.append(t)
                    return f

                attention_core(
                    qcT, kvc["kcT"], kvc["vc"], caT, False,
                    [mk_w1(g) for g in range(4)],
                )
                tap(f"ca{l}", caT)
                x2 = layer_norm(residual(x1, caT))
                tap(f"x2_{l}", x2)

                y3 = ffn_block(l, x2, w1g)
                tap(f"ff{l}", y3)
                xT = layer_norm(y3, out_f32=(l == n_layers - 1))

            if stage != "full":
                xf = xopool.tile([P, NSUB, S], f32, tag="xo")
                nc.vector.tensor_copy(xf[:], xT[:])
                xT = xf
            nc.sync.dma_start(out_d[:], xT[:])

    nc.compile()
    return nc


def _prep_fast(inputs, n_layers):
    """Host-side marshalling for the fast path (no biases, no gammas)."""
    g = {}

    def wqk_prep(w):  # [NL, H, D, DK] -> [nl, P, NSUB, D]
        out = np.empty((n_layers, P, NSUB, D), np.float32)
        for l in range(n_layers):
            w2d = np.asarray(w[l], np.float32).transpose(1, 0, 2).reshape(D, H * DK)
            out[l] = w2d.reshape(NSUB, P, H * DK).transpose(1, 0, 2)
        return np.ascontiguousarray(out)

    g["wq_s"] = wqk_prep(inputs["Wq_s"])
    g["wk_s"] = wqk_prep(inputs["Wk_s"])
    g["wv_s"] = wqk_prep(inputs["Wv_s"])
    g["wq_c"] = wqk_prep(inputs["Wq_c"])
    g["wk_c"] = wqk_prep(inputs["Wk_c"])
    g["wv_c"] = wqk_prep(inputs["Wv_c"])

    w1 = np.empty((n_layers, P, NSUB, DFF), np.float32)
    w2 = np.empty((n_layers, P, NF, D), np.float32)
    for l in range(n_layers):
        w1[l] = (
            np.asarray(inputs["W1"][l], np.float32)
            .reshape(NSUB, P, DFF)
            .transpose(1, 0, 2)
        )
        w2[l] = (
            np.asarray(inputs["W2"][l], np.float32)
            .reshape(NF, P, D)
            .transpose(1, 0, 2)
        )
    g["w1"] = np.ascontiguousarray(w1)
    g["w2"] = np.ascontiguousarray(w2)

    g["ones_row"] = np.ones((1, S), np.float32)
    g["invD_col"] = np.full((P, 1), 1.0 / D, np.float32)
    q = np.arange(P)
    g["tri01"] = (q[None, :] >= q[:, None]).astype(np.float32)
    g["vones"] = np.ones((P, NCH, H, 1), np.float32)
    return g


def _fast_path_ok(inputs):
    z = lambda k: not bool(np.any(np.asarray(inputs[k])))
    one = lambda k: bool(np.all(np.asarray(inputs[k]) == 1.0))
    return (
        z("bq_s") and z("bk_s") and z("bv_s")
        and z("bq_c") and z("bk_c") and z("bv_c")
        and z("b1") and z("b2")
        and one("ln1_g") and z("ln1_b") and one("ln2_g") and z("ln2_b")
    )


def _prep_shared(inputs, n_layers):
    """Host-side marshalling of weights into device tile layouts (float32;
    kernel() casts matmul-side arrays to the MM_DT numpy dtype)."""
    g = {}
    emb = np.asarray(inputs["emb"], np.float32)

    def wqk_prep(w):  # [NL, H, D, DK] -> [nl, P, NSUB, D]
        out = np.empty((n_layers, P, NSUB, D), np.float32)
        for l in range(n_layers):
            w2d = np.asarray(w[l], np.float32).transpose(1, 0, 2).reshape(D, H * DK)
            out[l] = w2d.reshape(NSUB, P, H * DK).transpose(1, 0, 2)
        return np.ascontiguousarray(out)

    def wv_prep(w, bv):  # augmented: per head 64 v-cols + ones col
        wout = np.empty((n_layers, P, NSUB, HW_COLS), np.float32)
        brow = np.zeros((n_layers, 1, HW_COLS), np.float32)
        for l in range(n_layers):
            aug = np.zeros((D, HW_COLS), np.float32)
            baug = np.zeros(HW_COLS, np.float32)
            wl = np.asarray(w[l], np.float32)  # [H, D, DVh]
            bl = np.asarray(bv[l], np.float32)  # [H, DVh]
            for h in range(H):
                aug[:, h * 65 : h * 65 + 64] = wl[h]
                baug[h * 65 : h * 65 + 64] = bl[h]
                baug[h * 65 + 64] = 1.0
            wout[l] = aug.reshape(NSUB, P, HW_COLS).transpose(1, 0, 2)
            brow[l, 0] = baug
        return np.ascontiguousarray(wout), brow

    def bcol_prep(b):  # [NL, ...] -> [nl, P, width]
        out = np.stack(
            [_col_layout(np.asarray(b[l], np.float32)) for l in range(n_layers)]
        )
        return np.ascontiguousarray(out)

    g["wq_s"] = wqk_prep(inputs["Wq_s"])
    g["wk_s"] = wqk_prep(inputs["Wk_s"])
    g["wv_s"], g["bv_s"] = wv_prep(inputs["Wv_s"], inputs["bv_s"])
    g["bq_s"] = bcol_prep(inputs["bq_s"])
    g["bk_s"] = bcol_prep(inputs["bk_s"])
    g["wq_c"] = wqk_prep(inputs["Wq_c"])
    g["wk_c"] = wqk_prep(inputs["Wk_c"])
    g["wv_c"], g["bv_c"] = wv_prep(inputs["Wv_c"], inputs["bv_c"])
    g["bq_c"] = bcol_prep(inputs["bq_c"])
    g["bk_c"] = bcol_prep(inputs["bk_c"])

    w1 = np.empty((n_layers, P, NSUB, DFF), np.float32)
    w2 = np.empty((n_layers, P, NF, D), np.float32)
    for l in range(n_layers):
        w1[l] = (
            np.asarray(inputs["W1"][l], np.float32)
            .reshape(NSUB, P, DFF)
            .transpose(1, 0, 2)
        )
        w2[l] = (
            np.asarray(inputs["W2"][l], np.float32)
            .reshape(NF, P, D)
            .transpose(1, 0, 2)
        )
    g["w1"] = np.ascontiguousarray(w1)
    g["w2"] = np.ascontiguousarray(w2)
    g["b1c"] = bcol_prep(inputs["b1"])
    g["b2c"] = bcol_prep(inputs["b2"])
    g["ln1g"] = bcol_prep(inputs["ln1_g"])
    g["ln1b"] = bcol_prep(inputs["ln1_b"])
    g["ln2g"] = bcol_prep(inputs["ln2_g"])
    g["ln2b"] = bcol_prep(inputs["ln2_b"])

    g["peT"] = _to_T_tiles(_pe_table())
    g["ones_row"] = np.ones((1, S), np.float32)
    g["invD_col"] = np.full((P, 1), 1.0 / D, np.float32)
    sel2 = np.zeros((2, P), np.float32)
    sel2[0, 0:64] = 1.0
    sel2[1, 64:128] = 1.0
    g["sel2"] = sel2
    q = np.arange(P)
    g["tri01"] = (q[None, :] >= q[:, None]).astype(np.float32)
    g["ident"] = np.eye(P, dtype=np.float32)
    return g, emb


def _mask_T8(mask_b):
    """[S, S] additive mask -> [P, NCH, S] transposed, pre-scaled by 8."""
    m = np.ascontiguousarray(np.asarray(mask_b, np.float32).T) * 8.0
    return np.ascontiguousarray(m.reshape(NCH, P, S).transpose(1, 0, 2))


# f32 bias-column tensors; everything else carries the matmul dtype
_F32_KEYS = {
    "bq_s", "bk_s", "bq_c", "bk_c", "b1c", "b2c",
    "ln1g", "ln1b", "ln2g", "ln2b",
}


def kernel(**inputs):
    global LAST_RESULT
    _ensure_path()
    import ml_dtypes
    from concourse.bass_utils import run_bass_kernel_spmd

    n_layers = N_LAYERS
    mm_np = np.float32 if MM_DT == "f32r" else ml_dtypes.bfloat16
    ids = np.asarray(inputs["decoder_input"])
    enc = np.asarray(inputs["encoder_output"], np.float32)
    smask = np.asarray(inputs["self_mask"], np.float32)
    cmask = np.asarray(inputs["cross_mask"], np.float32)

    tril = np.tril(np.ones((S, S), bool))
    canon = np.where(tril, np.float32(0.0), np.float32(-1e9))
    causal_self = all(np.array_equal(smask[b], canon) for b in range(B))
    self_needs_mask = (not causal_self) and bool(np.any(smask != 0.0))
    cross_needs_mask = bool(np.any(cmask != 0.0))

    if causal_self and not cross_needs_mask and _fast_path_ok(inputs):
        emb = np.asarray(inputs["emb"], np.float32)
        shared = _prep_fast(inputs, n_layers)
        shared = {k: v.astype(mm_np) for k, v in shared.items()}
        key = ("fast", n_layers, tuple(TAPS), MM_DT, STAGE)
        if key not in _BUILD_CACHE:
            _BUILD_CACHE[key] = _build_fast(n_layers, tuple(TAPS), MM_DT, STAGE)
        nc = _BUILD_CACHE[key]
        pe = _pe_table()
        in_maps = []
        for b in range(B):
            m = dict(shared)
            m["x0T"] = _to_T_tiles(emb[ids[b]] + pe).astype(mm_np)
            m["encT"] = _to_T_tiles(enc[b]).astype(mm_np)
            in_maps.append(m)
        res = run_bass_kernel_spmd(nc, in_maps, core_ids=list(range(8)))
        LAST_RESULT = res
        out = np.empty((B, S, D), np.float32)
        for b in range(B):
            xt = np.asarray(res.results[b]["out_xT"], np.float32)
            out[b] = xt.transpose(1, 0, 2).reshape(D, S).T
        return out

    shared, emb = _prep_shared(inputs, n_layers)
    shared.pop("sel2", None)
    shared = {
        k: (v if k in _F32_KEYS else v.astype(mm_np)) for k, v in shared.items()
    }

    key = (n_layers, causal_self, self_needs_mask, cross_needs_mask, tuple(TAPS), MM_DT)
    if key not in _BUILD_CACHE:
        _BUILD_CACHE[key] = _build(
            n_layers, causal_self, self_needs_mask, cross_needs_mask, tuple(TAPS),
            MM_DT,
        )
    nc = _BUILD_CACHE[key]

    in_maps = []
    for b in range(B):
        m = dict(shared)
        m["x0T"] = _to_T_tiles(emb[ids[b]]).astype(mm_np)
        m["encT"] = _to_T_tiles(enc[b]).astype(mm_np)
        if self_needs_mask:
            m["smaskT8"] = _mask_T8(smask[b]).astype(mm_np)
        if cross_needs_mask:
            m["cmaskT8"] = _mask_T8(cmask[b]).astype(mm_np)
        if not causal_self:
            m.pop("tri01", None)
        if not (self_needs_mask or cross_needs_mask):
            m.pop("ident", None)
        in_maps.append(m)

    res = run_bass_kernel_spmd(nc, in_maps, core_ids=list(range(8)))
    LAST_RESULT = res

    out = np.empty((B, S, D), np.float32)
    for b in range(B):
        xt = np.asarray(res.results[b]["out_xT"], np.float32)  # [P, NSUB, S]
        out[b] = xt.transpose(1, 0, 2).reshape(D, S).T
    return out

